# revision 21
# baseline (speedup 1.0000x reference)
"""GQA (16 query heads, 4 KV groups) forward kernel for 8 Trainium2 NeuronCores.

Sharding: core = (batch b in 0..1) x (kv-group g in 0..3).  Each core owns one
batch element and one whole KV group (4 query heads), computing the output
slice out[b, :, g*512:(g+1)*512] (stored transposed in DRAM; host untransposes).

Per-core plan (all matmul inputs bf16, fp32 PSUM accumulation), tuned to keep
the PE continuously busy (TRN2 p-state: 2.4 GHz only after ~3us of
uninterrupted tensor-engine activity):
  - Input DMAs split across 4 issue queues (SP/DVE/ACT/Pool) in priority
    order so the K-projection inputs land first.
  - All projections (K, V, Q for all 4 q-chunks) emitted as 16-matmul PSUM
    chunks; V transposed to natural layout on PE.
  - Attention in transposed-score layout, processed per head-PAIR:
    one [128k x 1024] scores matmul per (pair, kb), one fused exp on ACT,
    causal mask via one fused gpsimd affine_select (diag blocks only),
    softmax partial sums accumulated in bf16 on DVE, P@V as one
    [128, 1024] matmul accumulating in PSUM.
  - Denominators: ones-matmul on PE reduces partials over partitions and
    broadcasts to all 128 partitions in one shot; DVE reciprocal +
    multiply normalizes PV straight out of PSUM; output stays [d, q]
    (transposed) and is fixed up on the host.
  - Projection chunks are interleaved into the attention loops as PE
    filler so the tensor engine never idles while ACT chews on exps.
"""

import sys

if "/opt/trn_rl_repo" not in sys.path:
    sys.path.insert(0, "/opt/trn_rl_repo")

import ml_dtypes
import numpy as np

import concourse.bass as bass
import concourse.mybir as mybir
import concourse.tile as tile
from concourse import bacc
from concourse.bass_utils import run_bass_kernel_spmd
from concourse.masks import make_identity

B, T, C = 2, 2048, 2048
HEADS, GROUPS = 16, 4
HD = C // HEADS          # 128 head dim
H2G = HEADS // GROUPS    # 4 query heads per group
DG = H2G * HD            # 512 output cols per core
DKV = HD                 # 128 kv dim per group
NCT = C // 128           # 16 contraction tiles
NQC = T // 512           # 4 query chunks
NKB = T // 128           # 16 key blocks
SCALE = HD ** -0.5

F32 = mybir.dt.float32
BF16 = mybir.dt.bfloat16


def _body(tc, xb, wqt, wkt, wvt, out_d):
    nc = tc.nc
    act_exp = mybir.ActivationFunctionType.Exp
    is_ge = mybir.AluOpType.is_ge

    with (
        tc.tile_pool(name="const", bufs=1) as cpool,
        tc.tile_pool(name="data", bufs=1) as data,
        tc.tile_pool(name="vstage", bufs=2) as vstg,
        tc.tile_pool(name="sums_p", bufs=2) as sums_pool,
        tc.tile_pool(name="ex_p", bufs=9) as expool,
        tc.tile_pool(name="osb_p", bufs=2) as osbp,
        tc.tile_pool(name="rv_p", bufs=2) as rvp,
        tc.tile_pool(name="attn_ps", bufs=2, space="PSUM") as aps,
        tc.tile_pool(name="pv_ps", bufs=2, space="PSUM") as pvp,
    ):
        id_b = cpool.tile([128, 128], BF16)
        make_identity(nc, id_b)
        ones_b = cpool.tile([128, 128], BF16)
        nc.vector.memset(ones_b[:], 1.0)

        xT = data.tile([128, NCT, T], BF16)    # x^T: [c%128, c//128, t]
        wq = data.tile([128, NCT, DG], BF16)   # Wq^T tiles [c%128, c//128, d]
        wk = data.tile([128, NCT, DKV], BF16)
        wv = data.tile([128, NCT, DKV], BF16)
        kT = data.tile([128, T], BF16)         # K^T: [d, t]
        vn = data.tile([128, NKB, DKV], BF16)  # V natural: [t%128, t//128, d]
        qt = data.tile([128, H2G, T], BF16)    # Q^T: [d, h, t]

        # ---- input DMAs: priority-ordered, split across 3 issue queues ----
        # One DMA per (x-tile ci, q-chunk tq): contiguous 1KB rows, 128KB
        # each, so transfers spread across the 16 DMA engines and the
        # first-needed tiles (wk + x tq0) land in ~10us.
        def x_dma(eng, ci, tq):
            eng.dma_start(
                out=xT[:, ci, tq * 512:(tq + 1) * 512],
                in_=xb[ci * 128:(ci + 1) * 128, tq * 512:(tq + 1) * 512],
            )

        def wk_dma(eng, g4, w_dram, w_tile):
            src = w_dram[g4 * 512:(g4 + 1) * 512, :].rearrange(
                "(ci p) d -> p ci d", p=128)
            eng.dma_start(out=w_tile[:, 4 * g4:4 * g4 + 4, :], in_=src)

        def wq_dma(eng, ci):
            eng.dma_start(out=wq[:, ci, :],
                          in_=wqt[ci * 128:(ci + 1) * 128, :])

        # SP queue
        wk_dma(nc.sync, 0, wkt, wk)
        wk_dma(nc.sync, 1, wkt, wk)
        for ci in range(0, 5):
            x_dma(nc.sync, ci, 0)
        for g4 in range(4):
            wk_dma(nc.sync, g4, wvt, wv)
        for ci in range(12, 16):
            wq_dma(nc.sync, ci)
        for ci in range(0, 5):
            x_dma(nc.sync, ci, 1)
        # ACT queue
        wk_dma(nc.scalar, 2, wkt, wk)
        wk_dma(nc.scalar, 3, wkt, wk)
        for ci in range(5, 10):
            x_dma(nc.scalar, ci, 0)
        for ci in range(0, 8):
            wq_dma(nc.scalar, ci)
        for ci in range(5, 10):
            x_dma(nc.scalar, ci, 1)
        # Pool (gpsimd) queue
        for ci in range(10, 16):
            x_dma(nc.gpsimd, ci, 0)
        for ci in range(8, 12):
            wq_dma(nc.gpsimd, ci)
        for ci in range(10, 16):
            x_dma(nc.gpsimd, ci, 1)

        # ---- PE warm-up: ramp the tensor-engine p-state while DMAs land.
        # Narrow (128-col) matmuls: the DVFS ramp needs busy TIME, not work.
        warm = cpool.tile([128, 128], BF16)
        nc.vector.memset(warm[:], 0.0)
        for w in range(44):
            wps = aps.tile([128, 128], F32, tag="st", name=f"warm{w}")
            nc.tensor.matmul(wps[:], id_b[:], warm[:], start=True, stop=True)

        # ---- projection chunk emitters (each: 16 matmuls + PSUM->SBUF) ----
        def emit_k_chunk(tq):
            ps = aps.tile([128, 512], F32, tag="st", name=f"kps{tq}")
            for ci in range(NCT):
                nc.tensor.matmul(
                    ps[:], wk[:, ci, :], xT[:, ci, tq * 512:(tq + 1) * 512],
                    start=(ci == 0), stop=(ci == NCT - 1),
                )
            nc.vector.tensor_copy(kT[:, tq * 512:(tq + 1) * 512], ps[:])

        def emit_v_chunk(tq):
            ps = aps.tile([128, 512], F32, tag="st", name=f"vps{tq}")
            for ci in range(NCT):
                nc.tensor.matmul(
                    ps[:], wv[:, ci, :], xT[:, ci, tq * 512:(tq + 1) * 512],
                    start=(ci == 0), stop=(ci == NCT - 1),
                )
            vs = vstg.tile([128, 512], BF16, tag="vs", name=f"vs{tq}")
            nc.vector.tensor_copy(vs[:], ps[:])
            for j in range(4):
                pt = aps.tile([128, 128], BF16, tag="st", name=f"vt{tq}_{j}")
                nc.tensor.transpose(pt[:], vs[:, j * 128:(j + 1) * 128], id_b)
                nc.vector.tensor_copy(vn[:, 4 * tq + j, :], pt[:])

        def emit_q_chunk(tq, h):
            ps = aps.tile([128, 512], F32, tag="st", name=f"qps{tq}_{h}")
            for ci in range(NCT):
                nc.tensor.matmul(
                    ps[:],
                    wq[:, ci, h * 128:(h + 1) * 128],
                    xT[:, ci, tq * 512:(tq + 1) * 512],
                    start=(ci == 0), stop=(ci == NCT - 1),
                )
            nc.vector.tensor_copy(qt[:, h, tq * 512:(tq + 1) * 512], ps[:])

        # ---- upfront projections: K0, V0, then Q chunk 0 ----
        emit_k_chunk(0)
        emit_v_chunk(0)
        for h in range(H2G):
            emit_q_chunk(0, h)

        # ---- attention: flat software-pipelined stream over all
        # (qc, head-pair, kb) steps with global lags so no engine's in-order
        # queue ever head-of-line blocks on the ACT exp chain:
        #   scores(n) -> pv(n-2) -> sums-add(n-3)
        # Projection-chunk fillers are pinned to specific global steps.
        steps = []
        pair_state = {}
        for qc in range(NQC):
            for pi in range(2):
                nkb = 4 * qc + 4
                for kb in range(nkb):
                    steps.append((qc, pi, kb))

        # deferred x(tq2)/x(tq3) DMA issues, throttled by the ACT/GP
        # instruction streams so their transfers don't steal HBM bandwidth
        # from the urgently-needed x(tq1)/wq
        dma_sched = {}
        for k, ci in enumerate(range(0, 8)):
            dma_sched.setdefault(k, []).append(
                lambda ci=ci: x_dma(nc.scalar, ci, 2))
        for k, ci in enumerate(range(8, 16)):
            dma_sched.setdefault(k, []).append(
                lambda ci=ci: x_dma(nc.gpsimd, ci, 2))
        for k, ci in enumerate(range(0, 8)):
            dma_sched.setdefault(8 + k, []).append(
                lambda ci=ci: x_dma(nc.scalar, ci, 3))
        for k, ci in enumerate(range(8, 16)):
            dma_sched.setdefault(8 + k, []).append(
                lambda ci=ci: x_dma(nc.gpsimd, ci, 3))

        filler_sched = {
            0: lambda: emit_k_chunk(1),
            2: lambda: emit_v_chunk(1),
            3: lambda: emit_q_chunk(1, 0),
            4: lambda: emit_q_chunk(1, 1),
            5: lambda: emit_q_chunk(1, 2),
            6: lambda: emit_q_chunk(1, 3),
            8: lambda: emit_k_chunk(2),
            10: lambda: emit_v_chunk(2),
            12: lambda: emit_q_chunk(2, 0),
            14: lambda: emit_q_chunk(2, 1),
            16: lambda: emit_q_chunk(2, 2),
            18: lambda: emit_q_chunk(2, 3),
            26: lambda: emit_q_chunk(3, 0),
            31: lambda: emit_q_chunk(3, 1),
            36: lambda: emit_q_chunk(3, 2),
            41: lambda: emit_q_chunk(3, 3),
            49: lambda: emit_k_chunk(3),
            51: lambda: emit_v_chunk(3),
        }

        def get_pair(qc, pi):
            key = (qc, pi)
            if key not in pair_state:
                if qc not in osb_tiles:
                    osb_tiles[qc] = osbp.tile([128, H2G, 512], F32,
                                              tag="osb", name=f"osb{qc}")
                pair_state[key] = {
                    "pv": pvp.tile([128, 2, 512], F32, tag="pv",
                                   name=f"pv{qc}_{pi}"),
                    "sums": sums_pool.tile([128, 2, 512], BF16, tag="sums",
                                           name=f"sums{qc}_{pi}"),
                    "ex": {},
                }
            return pair_state[key]

        osb_tiles = {}

        def q_lo(qc, kb):
            # causal column restriction: for diagonal block j = kb - 4*qc,
            # only q in [j*128, 512) can attend key block kb.
            return max(0, (kb - 4 * qc) * 128)

        def emit_scores(n):
            qc, pi, kb = steps[n]
            ps = get_pair(qc, pi)
            lo = q_lo(qc, kb)
            st = aps.tile([128, 2, 512], F32, tag="st", name=f"st{n}")
            for i in range(2):
                nc.tensor.matmul(
                    st[:, i, lo:],
                    kT[:, kb * 128:(kb + 1) * 128],
                    qt[:, 2 * pi + i, qc * 512 + lo:(qc + 1) * 512],
                    start=True, stop=True,
                )
            ex = expool.tile([128, 2, 512], BF16, tag="ex", name=f"ex{n}")
            nc.scalar.activation(ex[:, :, lo:], st[:, :, lo:], act_exp,
                                 scale=SCALE)
            if kb >= 4 * qc:
                # causal triangle within the restricted range: keep f >= p
                nc.gpsimd.affine_select(
                    out=ex[:, :, lo:], in_=ex[:, :, lo:],
                    compare_op=is_ge,
                    fill=0.0,
                    base=0,
                    pattern=[[0, 2], [1, 512 - lo]],
                    channel_multiplier=-1,
                )
            ps["ex"][kb] = ex

        def emit_pv(n):
            qc, pi, kb = steps[n]
            ps = get_pair(qc, pi)
            nkb = 4 * qc + 4
            lo = q_lo(qc, kb)
            ex = ps["ex"][kb]
            for i in range(2):
                nc.tensor.matmul(
                    ps["pv"][:, i, lo:], vn[:, kb, :], ex[:, i, lo:],
                    start=(kb == 0), stop=(kb == nkb - 1),
                    skip_group_check=True,
                )

        def emit_add(n):
            qc, pi, kb = steps[n]
            ps = get_pair(qc, pi)
            lo = q_lo(qc, kb)
            ex = ps["ex"].pop(kb)
            if kb == 0:
                nc.vector.tensor_copy(ps["sums"][:], ex[:])
            else:
                nc.vector.tensor_add(ps["sums"][:, :, lo:],
                                     ps["sums"][:, :, lo:], ex[:, :, lo:])
            if kb == 4 * qc + 3:
                # pair complete: denominators + normalize + store
                emit_wrapup(qc, pi)

        def emit_wrapup(qc, pi):
            ps = pair_state.pop((qc, pi))
            osb = osb_tiles[qc]
            for i in range(2):
                h = 2 * pi + i
                sb = aps.tile([128, 512], F32, tag="st", name=f"sb{qc}_{h}")
                nc.tensor.matmul(sb[:], ones_b[:], ps["sums"][:, i, :],
                                 start=True, stop=True)
                rv = rvp.tile([128, 512], F32, tag="rv", name=f"rv{qc}_{h}")
                nc.vector.reciprocal_approx_fast(rv[:], sb[:])
                nc.vector.tensor_mul(osb[:, h, :], ps["pv"][:, i, :], rv[:])
                nc.sync.dma_start(
                    out=out_d[h * 128:(h + 1) * 128,
                              qc * 512:(qc + 1) * 512],
                    in_=osb[:, h, :],
                )

        nsteps = len(steps)
        pv_at = {}
        add_at = {}
        for s in range(nsteps):
            pv_lag = 4 if s < 64 else 2
            pv_at.setdefault(s + pv_lag, []).append(s)
            add_at.setdefault(s + pv_lag + 1, []).append(s)
        for n in range(nsteps + 5):
            if n < nsteps:
                if n in dma_sched:
                    for f in dma_sched[n]:
                        f()
                if n in filler_sched:
                    filler_sched[n]()
                emit_scores(n)
            for s in pv_at.get(n, []):
                emit_pv(s)
            for s in add_at.get(n, []):
                emit_add(s)


def build_nc():
    # Bacc (not raw Bass): its finalize passes split multi-sem waits
    # (move_matmul_waits_to_ldweights / generate_event_semaphores) to meet the
    # 1-wait-per-instruction hardware constraint walrus enforces.
    nc = bacc.Bacc("TRN2", target_bir_lowering=False)
    # xb is x[b] pre-transposed on the host: [C, T] bf16
    xb = nc.declare_dram_parameter("xb", [C, T], BF16, isOutput=False)
    wqt = nc.declare_dram_parameter("wqt", [C, DG], BF16, isOutput=False)
    wkt = nc.declare_dram_parameter("wkt", [C, DKV], BF16, isOutput=False)
    wvt = nc.declare_dram_parameter("wvt", [C, DKV], BF16, isOutput=False)
    # out is stored transposed [d, t]; the host untransposes at assemble time
    out_d = nc.declare_dram_parameter("out", [DG, T], F32, isOutput=True)
    with tile.TileContext(nc) as tc:
        _body(tc, xb, wqt, wkt, wvt, out_d)
    nc.compile()
    return nc


def make_in_maps(x, Wq, Wk, Wv):
    bf = ml_dtypes.bfloat16
    in_maps = []
    for b in range(B):
        xb = np.ascontiguousarray(x[b].T).astype(bf)
        for g in range(GROUPS):
            in_maps.append({
                "xb": xb,
                "wqt": np.ascontiguousarray(Wq[g * DG:(g + 1) * DG].T).astype(bf),
                "wkt": np.ascontiguousarray(Wk[g * DKV:(g + 1) * DKV].T).astype(bf),
                "wvt": np.ascontiguousarray(Wv[g * DKV:(g + 1) * DKV].T).astype(bf),
            })
    return in_maps


def assemble(results):
    out = np.empty((B, T, C), np.float32)
    for i, res in enumerate(results):
        b, g = divmod(i, GROUPS)
        out[b, :, g * DG:(g + 1) * DG] = res["out"].T
    return out


def run(x, Wq, Wk, Wv, **spmd_kwargs):
    nc = build_nc()
    in_maps = make_in_maps(x, Wq, Wk, Wv)
    return run_bass_kernel_spmd(nc, in_maps, list(range(8)), **spmd_kwargs)


def kernel(x, Wq, Wk, Wv):
    return assemble(run(x, Wq, Wk, Wv).results)


# revision 22
# speedup vs baseline: 1.1483x; 1.1483x over previous
"""GQA (16 query heads, 4 KV groups) forward kernel for 8 Trainium2 NeuronCores.

Sharding: core = (batch b in 0..1) x (kv-group g in 0..3).  Each core owns one
batch element and one whole KV group (4 query heads), computing the output
slice out[b, :, g*512:(g+1)*512] (stored transposed in DRAM; host untransposes).

Per-core plan (all matmul inputs bf16, fp32 PSUM accumulation), tuned to keep
the PE continuously busy (TRN2 p-state: 2.4 GHz only after ~3us of
uninterrupted tensor-engine activity):
  - Input DMAs split across 4 issue queues (SP/DVE/ACT/Pool) in priority
    order so the K-projection inputs land first.
  - All projections (K, V, Q for all 4 q-chunks) emitted as 16-matmul PSUM
    chunks; V transposed to natural layout on PE.
  - Attention in transposed-score layout, processed per head-PAIR:
    one [128k x 1024] scores matmul per (pair, kb), one fused exp on ACT,
    causal mask via one fused gpsimd affine_select (diag blocks only),
    softmax partial sums accumulated in bf16 on DVE, P@V as one
    [128, 1024] matmul accumulating in PSUM.
  - Denominators: ones-matmul on PE reduces partials over partitions and
    broadcasts to all 128 partitions in one shot; DVE reciprocal +
    multiply normalizes PV straight out of PSUM; output stays [d, q]
    (transposed) and is fixed up on the host.
  - Projection chunks are interleaved into the attention loops as PE
    filler so the tensor engine never idles while ACT chews on exps.
"""

import sys

if "/opt/trn_rl_repo" not in sys.path:
    sys.path.insert(0, "/opt/trn_rl_repo")

import ml_dtypes
import numpy as np

import concourse.bass as bass
import concourse.mybir as mybir
import concourse.tile as tile
from concourse import bacc
from concourse.bass_utils import run_bass_kernel_spmd
from concourse.masks import make_identity

B, T, C = 2, 2048, 2048
HEADS, GROUPS = 16, 4
HD = C // HEADS          # 128 head dim
H2G = HEADS // GROUPS    # 4 query heads per group
DG = H2G * HD            # 512 output cols per core
DKV = HD                 # 128 kv dim per group
NCT = C // 128           # 16 contraction tiles
NQC = T // 512           # 4 query chunks
NKB = T // 128           # 16 key blocks
SCALE = HD ** -0.5

F32 = mybir.dt.float32
BF16 = mybir.dt.bfloat16


def _body(tc, xb, wqt, wkt, wvt, out_d):
    nc = tc.nc
    act_exp = mybir.ActivationFunctionType.Exp
    is_ge = mybir.AluOpType.is_ge

    with (
        tc.tile_pool(name="const", bufs=1) as cpool,
        tc.tile_pool(name="data", bufs=1) as data,
        tc.tile_pool(name="vstage", bufs=2) as vstg,
        tc.tile_pool(name="sums_p", bufs=2) as sums_pool,
        tc.tile_pool(name="ex_p", bufs=9) as expool,
        tc.tile_pool(name="osb_p", bufs=2) as osbp,
        tc.tile_pool(name="rv_p", bufs=2) as rvp,
        tc.tile_pool(name="attn_ps", bufs=2, space="PSUM") as aps,
        tc.tile_pool(name="pv_ps", bufs=2, space="PSUM") as pvp,
    ):
        id_b = cpool.tile([128, 128], BF16)
        make_identity(nc, id_b)
        ones_b = cpool.tile([128, 128], BF16)
        nc.vector.memset(ones_b[:], 1.0)

        xT = data.tile([128, NCT, T], BF16)    # x^T: [c%128, c//128, t]
        wq = data.tile([128, NCT, DG], BF16)   # Wq^T tiles [c%128, c//128, d]
        wk = data.tile([128, NCT, DKV], BF16)
        wv = data.tile([128, NCT, DKV], BF16)
        kT = data.tile([128, T], BF16)         # K^T: [d, t]
        vn = data.tile([128, NKB, DKV], BF16)  # V natural: [t%128, t//128, d]
        qt = data.tile([128, H2G, T], BF16)    # Q^T: [d, h, t]

        # ---- input DMAs: priority-ordered, split across 3 issue queues ----
        # One DMA per (x-tile ci, q-chunk tq): contiguous 1KB rows, 128KB
        # each, so transfers spread across the 16 DMA engines and the
        # first-needed tiles (wk + x tq0) land in ~10us.
        def x_dma(eng, ci, tq):
            eng.dma_start(
                out=xT[:, ci, tq * 512:(tq + 1) * 512],
                in_=xb[ci * 128:(ci + 1) * 128, tq * 512:(tq + 1) * 512],
            )

        def wk_dma(eng, g4, w_dram, w_tile):
            src = w_dram[g4 * 512:(g4 + 1) * 512, :].rearrange(
                "(ci p) d -> p ci d", p=128)
            eng.dma_start(out=w_tile[:, 4 * g4:4 * g4 + 4, :], in_=src)

        def wq_dma(eng, ci):
            eng.dma_start(out=wq[:, ci, :],
                          in_=wqt[ci * 128:(ci + 1) * 128, :])

        # SP queue
        wk_dma(nc.sync, 0, wkt, wk)
        wk_dma(nc.sync, 1, wkt, wk)
        for ci in range(0, 5):
            x_dma(nc.sync, ci, 0)
        for g4 in range(4):
            wk_dma(nc.sync, g4, wvt, wv)
        for ci in range(12, 16):
            wq_dma(nc.sync, ci)
        for tq in range(1, 3):
            for ci in range(0, 5):
                x_dma(nc.sync, ci, tq)
        # ACT queue
        wk_dma(nc.scalar, 2, wkt, wk)
        wk_dma(nc.scalar, 3, wkt, wk)
        for ci in range(5, 10):
            x_dma(nc.scalar, ci, 0)
        for ci in range(0, 8):
            wq_dma(nc.scalar, ci)
        for tq in range(1, 3):
            for ci in range(5, 10):
                x_dma(nc.scalar, ci, tq)
        # Pool (gpsimd) queue
        for ci in range(10, 16):
            x_dma(nc.gpsimd, ci, 0)
        for ci in range(8, 12):
            wq_dma(nc.gpsimd, ci)
        for tq in range(1, 3):
            for ci in range(10, 16):
                x_dma(nc.gpsimd, ci, tq)

        # ---- PE warm-up: ramp the tensor-engine p-state while DMAs land.
        # Narrow (128-col) matmuls: the DVFS ramp needs busy TIME, not work.
        warm = cpool.tile([128, 128], BF16)
        nc.vector.memset(warm[:], 0.0)
        for w in range(44):
            wps = aps.tile([128, 128], F32, tag="st", name=f"warm{w}")
            nc.tensor.matmul(wps[:], id_b[:], warm[:], start=True, stop=True)

        # ---- projection chunk emitters (each: 16 matmuls + PSUM->SBUF) ----
        def emit_k_chunk(tq):
            ps = aps.tile([128, 512], F32, tag="st", name=f"kps{tq}")
            for ci in range(NCT):
                nc.tensor.matmul(
                    ps[:], wk[:, ci, :], xT[:, ci, tq * 512:(tq + 1) * 512],
                    start=(ci == 0), stop=(ci == NCT - 1),
                )
            nc.vector.tensor_copy(kT[:, tq * 512:(tq + 1) * 512], ps[:])

        def emit_v_chunk(tq):
            ps = aps.tile([128, 512], F32, tag="st", name=f"vps{tq}")
            for ci in range(NCT):
                nc.tensor.matmul(
                    ps[:], wv[:, ci, :], xT[:, ci, tq * 512:(tq + 1) * 512],
                    start=(ci == 0), stop=(ci == NCT - 1),
                )
            vs = vstg.tile([128, 512], BF16, tag="vs", name=f"vs{tq}")
            nc.vector.tensor_copy(vs[:], ps[:])
            for j in range(4):
                pt = aps.tile([128, 128], BF16, tag="st", name=f"vt{tq}_{j}")
                nc.tensor.transpose(pt[:], vs[:, j * 128:(j + 1) * 128], id_b)
                nc.vector.tensor_copy(vn[:, 4 * tq + j, :], pt[:])

        def emit_q_chunk(tq, h):
            ps = aps.tile([128, 512], F32, tag="st", name=f"qps{tq}_{h}")
            for ci in range(NCT):
                nc.tensor.matmul(
                    ps[:],
                    wq[:, ci, h * 128:(h + 1) * 128],
                    xT[:, ci, tq * 512:(tq + 1) * 512],
                    start=(ci == 0), stop=(ci == NCT - 1),
                )
            nc.vector.tensor_copy(qt[:, h, tq * 512:(tq + 1) * 512], ps[:])

        # ---- upfront projections: K0, V0, then Q chunk 0 ----
        emit_k_chunk(0)
        emit_v_chunk(0)
        for h in range(H2G):
            emit_q_chunk(0, h)

        # ---- attention: flat software-pipelined stream over all
        # (qc, head-pair, kb) steps with global lags so no engine's in-order
        # queue ever head-of-line blocks on the ACT exp chain:
        #   scores(n) -> pv(n-2) -> sums-add(n-3)
        # Projection-chunk fillers are pinned to specific global steps.
        steps = []
        pair_state = {}
        for qc in range(NQC):
            for pi in range(2):
                nkb = 4 * qc + 4
                for kb in range(nkb):
                    steps.append((qc, pi, kb))

        # deferred x(tq2)/x(tq3) DMA issues, throttled by the ACT/GP
        # instruction streams so their transfers don't steal HBM bandwidth
        # from the urgently-needed x(tq1)/wq
        dma_sched = {}
        for k, ci in enumerate(range(0, 8)):
            dma_sched.setdefault(8 + k, []).append(
                lambda ci=ci: x_dma(nc.scalar, ci, 3))
        for k, ci in enumerate(range(8, 16)):
            dma_sched.setdefault(8 + k, []).append(
                lambda ci=ci: x_dma(nc.gpsimd, ci, 3))

        filler_sched = {
            0: lambda: emit_k_chunk(1),
            2: lambda: emit_v_chunk(1),
            3: lambda: emit_q_chunk(1, 0),
            4: lambda: emit_q_chunk(1, 1),
            5: lambda: emit_q_chunk(1, 2),
            6: lambda: emit_q_chunk(1, 3),
            8: lambda: emit_k_chunk(2),
            10: lambda: emit_v_chunk(2),
            12: lambda: emit_q_chunk(2, 0),
            14: lambda: emit_q_chunk(2, 1),
            16: lambda: emit_q_chunk(2, 2),
            18: lambda: emit_q_chunk(2, 3),
            26: lambda: emit_q_chunk(3, 0),
            31: lambda: emit_q_chunk(3, 1),
            36: lambda: emit_q_chunk(3, 2),
            41: lambda: emit_q_chunk(3, 3),
            49: lambda: emit_k_chunk(3),
            51: lambda: emit_v_chunk(3),
        }

        def get_pair(qc, pi):
            key = (qc, pi)
            if key not in pair_state:
                if qc not in osb_tiles:
                    osb_tiles[qc] = osbp.tile([128, H2G, 512], F32,
                                              tag="osb", name=f"osb{qc}")
                pair_state[key] = {
                    "pv": pvp.tile([128, 2, 512], F32, tag="pv",
                                   name=f"pv{qc}_{pi}"),
                    "sums": sums_pool.tile([128, 2, 512], BF16, tag="sums",
                                           name=f"sums{qc}_{pi}"),
                    "ex": {},
                }
            return pair_state[key]

        osb_tiles = {}

        def q_lo(qc, kb):
            # causal column restriction: for diagonal block j = kb - 4*qc,
            # only q in [j*128, 512) can attend key block kb.
            return max(0, (kb - 4 * qc) * 128)

        def emit_scores(n):
            qc, pi, kb = steps[n]
            ps = get_pair(qc, pi)
            lo = q_lo(qc, kb)
            st = aps.tile([128, 2, 512], F32, tag="st", name=f"st{n}")
            for i in range(2):
                nc.tensor.matmul(
                    st[:, i, lo:],
                    kT[:, kb * 128:(kb + 1) * 128],
                    qt[:, 2 * pi + i, qc * 512 + lo:(qc + 1) * 512],
                    start=True, stop=True,
                )
            ex = expool.tile([128, 2, 512], BF16, tag="ex", name=f"ex{n}")
            nc.scalar.activation(ex[:, :, lo:], st[:, :, lo:], act_exp,
                                 scale=SCALE)
            if kb >= 4 * qc:
                # causal triangle within the restricted range: keep f >= p
                nc.gpsimd.affine_select(
                    out=ex[:, :, lo:], in_=ex[:, :, lo:],
                    compare_op=is_ge,
                    fill=0.0,
                    base=0,
                    pattern=[[0, 2], [1, 512 - lo]],
                    channel_multiplier=-1,
                )
            ps["ex"][kb] = ex

        def emit_pv(n):
            qc, pi, kb = steps[n]
            ps = get_pair(qc, pi)
            nkb = 4 * qc + 4
            lo = q_lo(qc, kb)
            ex = ps["ex"][kb]
            for i in range(2):
                nc.tensor.matmul(
                    ps["pv"][:, i, lo:], vn[:, kb, :], ex[:, i, lo:],
                    start=(kb == 0), stop=(kb == nkb - 1),
                    skip_group_check=True,
                )

        def emit_add(n):
            qc, pi, kb = steps[n]
            ps = get_pair(qc, pi)
            lo = q_lo(qc, kb)
            ex = ps["ex"].pop(kb)
            if kb == 0:
                nc.vector.tensor_copy(ps["sums"][:], ex[:])
            else:
                nc.vector.tensor_add(ps["sums"][:, :, lo:],
                                     ps["sums"][:, :, lo:], ex[:, :, lo:])
            if kb == 4 * qc + 3:
                # pair complete: denominators + normalize + store
                emit_wrapup(qc, pi)

        def emit_wrapup(qc, pi):
            ps = pair_state.pop((qc, pi))
            osb = osb_tiles[qc]
            for i in range(2):
                h = 2 * pi + i
                sb = aps.tile([128, 512], F32, tag="st", name=f"sb{qc}_{h}")
                nc.tensor.matmul(sb[:], ones_b[:], ps["sums"][:, i, :],
                                 start=True, stop=True)
                rv = rvp.tile([128, 512], F32, tag="rv", name=f"rv{qc}_{h}")
                nc.vector.reciprocal_approx_fast(rv[:], sb[:])
                nc.vector.tensor_mul(osb[:, h, :], ps["pv"][:, i, :], rv[:])
                nc.sync.dma_start(
                    out=out_d[h * 128:(h + 1) * 128,
                              qc * 512:(qc + 1) * 512],
                    in_=osb[:, h, :],
                )

        nsteps = len(steps)
        pv_at = {}
        add_at = {}
        for s in range(nsteps):
            pv_lag = 4 if s < 64 else 2
            pv_at.setdefault(s + pv_lag, []).append(s)
            add_at.setdefault(s + pv_lag + 1, []).append(s)
        for n in range(nsteps + 5):
            if n < nsteps:
                if n in dma_sched:
                    for f in dma_sched[n]:
                        f()
                if n in filler_sched:
                    filler_sched[n]()
                emit_scores(n)
            for s in pv_at.get(n, []):
                emit_pv(s)
            for s in add_at.get(n, []):
                emit_add(s)


def build_nc():
    # Bacc (not raw Bass): its finalize passes split multi-sem waits
    # (move_matmul_waits_to_ldweights / generate_event_semaphores) to meet the
    # 1-wait-per-instruction hardware constraint walrus enforces.
    nc = bacc.Bacc("TRN2", target_bir_lowering=False)
    # xb is x[b] pre-transposed on the host: [C, T] bf16
    xb = nc.declare_dram_parameter("xb", [C, T], BF16, isOutput=False)
    wqt = nc.declare_dram_parameter("wqt", [C, DG], BF16, isOutput=False)
    wkt = nc.declare_dram_parameter("wkt", [C, DKV], BF16, isOutput=False)
    wvt = nc.declare_dram_parameter("wvt", [C, DKV], BF16, isOutput=False)
    # out is stored transposed [d, t]; the host untransposes at assemble time
    out_d = nc.declare_dram_parameter("out", [DG, T], F32, isOutput=True)
    with tile.TileContext(nc) as tc:
        _body(tc, xb, wqt, wkt, wvt, out_d)
    nc.compile()
    return nc


def make_in_maps(x, Wq, Wk, Wv):
    bf = ml_dtypes.bfloat16
    in_maps = []
    for b in range(B):
        xb = np.ascontiguousarray(x[b].T).astype(bf)
        for g in range(GROUPS):
            in_maps.append({
                "xb": xb,
                "wqt": np.ascontiguousarray(Wq[g * DG:(g + 1) * DG].T).astype(bf),
                "wkt": np.ascontiguousarray(Wk[g * DKV:(g + 1) * DKV].T).astype(bf),
                "wvt": np.ascontiguousarray(Wv[g * DKV:(g + 1) * DKV].T).astype(bf),
            })
    return in_maps


def assemble(results):
    out = np.empty((B, T, C), np.float32)
    for i, res in enumerate(results):
        b, g = divmod(i, GROUPS)
        out[b, :, g * DG:(g + 1) * DG] = res["out"].T
    return out


def run(x, Wq, Wk, Wv, **spmd_kwargs):
    nc = build_nc()
    in_maps = make_in_maps(x, Wq, Wk, Wv)
    return run_bass_kernel_spmd(nc, in_maps, list(range(8)), **spmd_kwargs)


def kernel(x, Wq, Wk, Wv):
    return assemble(run(x, Wq, Wk, Wv).results)
